# revision 9
# baseline (speedup 1.0000x reference)
"""MoE MLP (top-2 routing, capacity 1.25) on 8 Trainium2 NeuronCores.

Strategy (expert-parallel, per the sharding hint):
  - Router + top-k + capacity assignment run on host in float64 (cheap:
    0.27 GFLOP vs 344 GFLOP for the expert FFNs, and data-dependent
    control flow is a poor fit for the static Bass dataflow graph).
  - Every expert overflows capacity for this problem size (mean load
    4096 assignments vs cap 2560), so each of the 8 cores computes a
    dense [cap,D] @ [D,F] -> gelu -> [cap,F] @ [F,D] FFN for one expert.
  - Dispatch/combine (gather/scatter by routing indices) run on host.

Device kernel: bf16 matmuls with both weight stacks resident in SBUF
(16 MB, fits), activations feature-major ([D,cap]/[F,cap]) so weight
tiles are always the stationary operand and no transposes are needed.
The 2560-MM stream runs at the warm PE issue rate (~216 ns/MM at
N=512), so everything else is startup/tail engineering:
  - all streamed tensors are host-prepacked into layouts where every
    DMA is linear with >=2 KB per-partition runs (small-packet DMA ran
    at ~175 GB/s vs ~350 GB/s for big packets),
  - startup-critical loads are split across both HWDGE queues (Sync +
    Activation) so x(0) and the first w1 column-blocks land in
    parallel; issue order keeps every later block ahead of the PE,
  - biases go over HWDGE as host-pretransposed [P, n] linear rows (the
    SWDGE path delivered them ~18 us in, stalling the first gelu),
  - junk warmup matmuls bridge the preamble+first-DMA window so HAM
    reaches 2.4 GHz with no re-throttle,
  - the final psum group is split in quarters so only ~1 us of serial
    ACT+DMA trails the last matmul.
bf16 keeps HBM traffic at ~32 MB/core (vs 190 MB for the f32r
variant), which also stays under the board power limit - the f32r run
spent 161 us GPIO-throttled to 1.95 GHz.  rel err ~3.4e-3 (gate 2e-2).
"""

import numpy as np
import ml_dtypes

B, T, D, F_FF, E, TOP_K = 8, 2048, 1024, 4096, 8, 2
N = B * T
CAP = 2560          # int(1.25 * N / E)
NCORES = 8
P = 128
DC = D // P         # 8 chunks of the model dim
FC = F_FF // P      # 32 chunks of the ff dim
NT = 512            # token tile (one PSUM bank of fp32)
TT = CAP // NT      # 5 token tiles

BF16 = ml_dtypes.bfloat16

MODE = "bf16"       # kept for test.py compatibility; single path now

_NC_CACHE = {}


def _build_nc():
    """Per-core Bass graph: dense bf16 FFN for one expert (SPMD x8)."""
    from contextlib import ExitStack

    import concourse.mybir as mybir
    import concourse.tile as tile
    from concourse import bacc

    bf = mybir.dt.bfloat16
    f32 = mybir.dt.float32
    AF = mybir.ActivationFunctionType

    nc = bacc.Bacc(trn_type="TRN2")
    # host-prepacked layouts (all loads below are linear copies):
    #   xT[t,p,c,n]  = x.T chunk: token tile t, d-row c*128+p, token n
    #   w1[fb,p,c,j] = w1[c*128+p, fb*128+j]   (128-col blocks of [D,F])
    #   w2[dc,p,f,j] = w2[f*128+p, dc*128+j]   (128-col blocks of [F,D])
    #   b1[p,fc] = b1[fc*128+p]; b2[p,dc] = b2[dc*128+p]
    xT = nc.dram_tensor("xT", [TT, P, DC, NT], bf, kind="ExternalInput").ap()
    w1 = nc.dram_tensor("w1", [FC, P, DC, P], bf, kind="ExternalInput").ap()
    w2 = nc.dram_tensor("w2", [DC, P, FC, P], bf, kind="ExternalInput").ap()
    b1 = nc.dram_tensor("b1", [P, FC], f32, kind="ExternalInput").ap()
    b2 = nc.dram_tensor("b2", [P, DC], f32, kind="ExternalInput").ap()
    # bf16 output: halves the writeback DMA (host combines in fp32;
    # the extra ~0.2% quantization is far inside the 2e-2 gate)
    out = nc.dram_tensor("out", [D, CAP], bf, kind="ExternalOutput").ap()

    with tile.TileContext(nc) as tc, ExitStack() as ctx:
        cpool = ctx.enter_context(tc.tile_pool(name="consts", bufs=1))
        wpool = ctx.enter_context(tc.tile_pool(name="weights", bufs=1))
        xpool = ctx.enter_context(tc.tile_pool(name="xin", bufs=2))
        hpool = ctx.enter_context(tc.tile_pool(name="hmid", bufs=1))
        ypool = ctx.enter_context(tc.tile_pool(name="yout", bufs=4))
        # ph + py are distinct tags; 4 bufs each = all 8 PSUM banks
        ppool = ctx.enter_context(tc.tile_pool(name="psum", bufs=4, space="PSUM"))

        # PE warm-up: junk matmuls bridging the ~6us engine preamble plus
        # the first-DMA latency so the HAM clock-gate reaches 2.4 GHz and
        # the real stream starts warm with no idle gap (>~3.4us idle
        # would drop PE back to 1.2 GHz).
        warm = cpool.tile([P, NT], bf)
        nc.vector.memset(warm, 0.0)
        pwarm = ppool.tile([P, NT], f32, tag="ph")
        for _ in range(12):
            nc.tensor.matmul(pwarm, lhsT=warm[:, :P], rhs=warm, start=True,
                             stop=True)

        def load_x(t):
            xs = xpool.tile([P, DC, NT], bf, name=f"x_s{t}", tag="xs")
            nc.sync.dma_start(out=xs, in_=xT[t])
            return xs

        # Startup-critical DMAs split across the two HWDGE queues so
        # they land in parallel (each queue caps at ~150-195 GB/s while
        # ramping; together ~340).  The first matmul group needs all of
        # x(0) plus w1 block 0, so x(0) is split across the queues
        # roughly by their measured rates, and w1 block 0 leads the
        # Scalar queue (the tiny strided bias loads delayed that
        # queue's first packet by ~2us when they went first).  Block k
        # of w1 is consumed ~1.73us after block k-1 but transfers in
        # ~1.3us on the Sync queue alone, so the stream never waits
        # after the first group.
        x_cur = xpool.tile([P, DC, NT], bf, name="x_s0", tag="xs")
        XS = 6  # x(0) d-chunks on the Sync queue (rest on Scalar)
        w1_s = wpool.tile([P, FC, DC, P], bf)
        nc.sync.dma_start(out=x_cur[:, :XS, :], in_=xT[0, :, :XS, :])
        nc.scalar.dma_start(out=w1_s[:, 0], in_=w1[0])
        nc.scalar.dma_start(out=x_cur[:, XS:, :], in_=xT[0, :, XS:, :])
        # biases are 128 tiny descriptors each - park them on Sync right
        # after x(0) (done ~11.5us, first gelu needs b1 at ~14) so they
        # don't delay the Scalar queue's w1 blocks
        b1_s = cpool.tile([P, FC], f32)
        nc.sync.dma_start(out=b1_s, in_=b1)
        b2_s = cpool.tile([P, DC], f32)
        nc.sync.dma_start(out=b2_s, in_=b2)
        nc.sync.dma_start(out=w1_s[:, 1], in_=w1[1])
        nc.scalar.dma_start(out=w1_s[:, 2], in_=w1[2])
        nc.sync.dma_start(out=w1_s[:, 3], in_=w1[3])
        nc.scalar.dma_start(out=w1_s[:, 4], in_=w1[4])
        nc.sync.dma_start(out=w1_s[:, 5], in_=w1[5])
        nc.scalar.dma_start(out=w1_s[:, 6], in_=w1[6])
        for fb in range(7, FC):
            nc.sync.dma_start(out=w1_s[:, fb], in_=w1[fb])
        w2_s = wpool.tile([P, DC, FC, P], bf)
        for dc in range(DC):
            nc.sync.dma_start(out=w2_s[:, dc], in_=w2[dc])

        for t in range(TT):
            x_s = x_cur
            if t + 1 < TT:
                x_cur = load_x(t + 1)
            # h.T tile [f, tok] for this token tile
            h_s = hpool.tile([P, FC, NT], bf)
            for fc in range(FC):
                ph = ppool.tile([P, NT], f32, tag="ph")
                for c in range(DC):
                    nc.tensor.matmul(
                        ph,
                        lhsT=w1_s[:, fc, c, :],
                        rhs=x_s[:, c, :],
                        start=(c == 0),
                        stop=(c == DC - 1),
                    )
                nc.scalar.activation(
                    h_s[:, fc, :], ph, AF.Gelu, bias=b1_s[:, fc:fc + 1]
                )
            for dc in range(DC):
                # split the kernel's final group so less serial ACT+DMA
                # trails the last matmul (the trailing chain scales with
                # the last part's width)
                last = t == TT - 1 and dc == DC - 1
                widths = [192, 192, 64, 64] if last else [NT]
                off = 0
                for s, w in enumerate(widths):
                    py = ppool.tile([P, NT], f32, name="py", tag="py")
                    for fc in range(FC):
                        nc.tensor.matmul(
                            py[:, :w],
                            lhsT=w2_s[:, dc, fc, :],
                            rhs=h_s[:, fc, off:off + w],
                            start=(fc == 0),
                            stop=(fc == FC - 1),
                        )
                    y_s = ypool.tile([P, NT], bf, name="y_s", tag="ys")
                    nc.scalar.activation(
                        y_s[:, :w], py[:, :w], AF.Identity,
                        bias=b2_s[:, dc:dc + 1]
                    )
                    # final group: alternate queues so its writeback
                    # drains in parallel; last part issues on the same
                    # engine as its ACT (no cross-engine sem hop)
                    eng = nc.scalar if (last and s % 2 == 1) else nc.sync
                    eng.dma_start(
                        out=out[dc * P:(dc + 1) * P,
                                t * NT + off:t * NT + off + w],
                        in_=y_s[:, :w],
                    )
                    off += w
    nc.compile()
    return nc


def _route(x, w_router, b_router):
    """Replicates reference routing (softmax -> top-2 -> capacity) in f64.

    Returns per-expert (token_ids, slot_positions, gate_values)."""
    xf = x.reshape(N, D).astype(np.float64)
    logits = xf @ w_router.astype(np.float64) + b_router.astype(np.float64)
    logits -= logits.max(axis=-1, keepdims=True)
    p = np.exp(logits)
    gates = p / p.sum(axis=-1, keepdims=True)
    # top-2, ties to the lower index (matches lax.top_k)
    order = np.argsort(-gates, axis=1, kind="stable")[:, :TOP_K]
    topv = np.take_along_axis(gates, order, axis=1)
    e_flat = order.reshape(-1)
    g_flat = topv.reshape(-1).astype(np.float32)
    tok = np.repeat(np.arange(N), TOP_K)
    pos = np.empty(N * TOP_K, np.int64)
    for e in range(E):
        m_e = e_flat == e
        pos[m_e] = np.arange(int(m_e.sum()))
    keep = pos < CAP
    per_expert = []
    for e in range(E):
        sel = (e_flat == e) & keep
        per_expert.append((tok[sel], pos[sel], g_flat[sel]))
    return per_expert


def _run_device(in_maps, trace=False):
    from concourse.bass_utils import run_bass_kernel_spmd

    if "nc" not in _NC_CACHE:
        _NC_CACHE["nc"] = _build_nc()
    return run_bass_kernel_spmd(
        _NC_CACHE["nc"], in_maps, core_ids=list(range(NCORES)), trace=trace
    )


def _kernel_impl(inputs, trace=False):
    x = np.asarray(inputs["x"], dtype=np.float32)
    w_router = np.asarray(inputs["w_router"], dtype=np.float32)
    b_router = np.asarray(inputs["b_router"], dtype=np.float32)
    w1 = np.asarray(inputs["w1"], dtype=np.float32)
    b1 = np.ascontiguousarray(np.asarray(inputs["b1"], dtype=np.float32))
    w2 = np.asarray(inputs["w2"], dtype=np.float32)
    b2 = np.ascontiguousarray(np.asarray(inputs["b2"], dtype=np.float32))

    per_expert = _route(x, w_router, b_router)
    xf = x.reshape(N, D)

    in_maps = []
    for e in range(E):
        tk, ps, _ = per_expert[e]
        buf = np.zeros((CAP, D), np.float32)
        buf[ps] = xf[tk]
        bufT = buf.T                                # [D, CAP]
        # stream-block layouts (see _build_nc)
        xp = np.ascontiguousarray(
            bufT.reshape(DC, P, TT, NT).transpose(2, 1, 0, 3)).astype(BF16)
        w1p = np.ascontiguousarray(
            w1[e].reshape(DC, P, FC, P).transpose(2, 1, 0, 3)).astype(BF16)
        w2p = np.ascontiguousarray(
            w2[e].reshape(FC, P, DC, P).transpose(2, 1, 0, 3)).astype(BF16)
        b1p = np.ascontiguousarray(b1[e].reshape(FC, P).T)
        b2p = np.ascontiguousarray(b2[e].reshape(DC, P).T)
        in_maps.append({
            "xT": xp, "w1": w1p, "w2": w2p, "b1": b1p, "b2": b2p,
        })

    res = _run_device(in_maps, trace=trace)

    y = np.zeros((N, D), np.float32)
    ws = np.zeros((N,), np.float32)
    for e in range(E):
        tk, ps, gv = per_expert[e]
        outT = np.asarray(res.results[e]["out"]).astype(np.float32)  # [D, CAP]
        vals = (outT[:, ps] * gv[None, :]).T  # [n_e, D]
        y[tk] += vals                         # tk unique within one expert
        ws[tk] += gv
    y = np.where((ws > 0.0)[:, None], y / np.maximum(ws, 1e-6)[:, None], y)
    return y.reshape(B, T, D).astype(np.float32), res


def kernel(**inputs):
    y, _ = _kernel_impl(inputs, trace=False)
    return y


# revision 10
# speedup vs baseline: 1.0084x; 1.0084x over previous
"""MoE MLP (top-2 routing, capacity 1.25) on 8 Trainium2 NeuronCores.

Strategy (expert-parallel, per the sharding hint):
  - Router + top-k + capacity assignment run on host in float64 (cheap:
    0.27 GFLOP vs 344 GFLOP for the expert FFNs, and data-dependent
    control flow is a poor fit for the static Bass dataflow graph).
  - Every expert overflows capacity for this problem size (mean load
    4096 assignments vs cap 2560), so each of the 8 cores computes a
    dense [cap,D] @ [D,F] -> gelu -> [cap,F] @ [F,D] FFN for one expert.
  - Dispatch/combine (gather/scatter by routing indices) run on host.

Device kernel: bf16 matmuls with both weight stacks resident in SBUF
(16 MB, fits), activations feature-major ([D,cap]/[F,cap]) so weight
tiles are always the stationary operand and no transposes are needed.
The 2560-MM stream runs at the warm PE issue rate (~216 ns/MM at
N=512), so everything else is startup/tail engineering:
  - all streamed tensors are host-prepacked into layouts where every
    DMA is linear with >=2 KB per-partition runs (small-packet DMA ran
    at ~175 GB/s vs ~350 GB/s for big packets),
  - startup-critical loads are split across both HWDGE queues (Sync +
    Activation) so x(0) and the first w1 column-blocks land in
    parallel; issue order keeps every later block ahead of the PE,
  - biases go over HWDGE as host-pretransposed [P, n] linear rows (the
    SWDGE path delivered them ~18 us in, stalling the first gelu),
  - junk warmup matmuls bridge the preamble+first-DMA window so HAM
    reaches 2.4 GHz with no re-throttle,
  - the final psum group is split in quarters so only ~1 us of serial
    ACT+DMA trails the last matmul.
bf16 keeps HBM traffic at ~32 MB/core (vs 190 MB for the f32r
variant), which also stays under the board power limit - the f32r run
spent 161 us GPIO-throttled to 1.95 GHz.  rel err ~3.4e-3 (gate 2e-2).
"""

import numpy as np
import ml_dtypes

B, T, D, F_FF, E, TOP_K = 8, 2048, 1024, 4096, 8, 2
N = B * T
CAP = 2560          # int(1.25 * N / E)
NCORES = 8
P = 128
DC = D // P         # 8 chunks of the model dim
FC = F_FF // P      # 32 chunks of the ff dim
NT = 512            # token tile (one PSUM bank of fp32)
TT = CAP // NT      # 5 token tiles

BF16 = ml_dtypes.bfloat16

MODE = "bf16"       # kept for test.py compatibility; single path now

_NC_CACHE = {}


def _build_nc():
    """Per-core Bass graph: dense bf16 FFN for one expert (SPMD x8)."""
    from contextlib import ExitStack

    import concourse.mybir as mybir
    import concourse.tile as tile
    from concourse import bacc

    bf = mybir.dt.bfloat16
    f32 = mybir.dt.float32
    AF = mybir.ActivationFunctionType

    nc = bacc.Bacc(trn_type="TRN2")
    # host-prepacked layouts (all loads below are linear copies):
    #   xT[t,p,c,n]  = x.T chunk: token tile t, d-row c*128+p, token n
    #   w1[fb,p,c,j] = w1[c*128+p, fb*128+j]   (128-col blocks of [D,F])
    #   w2[dc,p,f,j] = w2[f*128+p, dc*128+j]   (128-col blocks of [F,D])
    #   b1[p,fc] = b1[fc*128+p]; b2[p,dc] = b2[dc*128+p]
    xT = nc.dram_tensor("xT", [TT, P, DC, NT], bf, kind="ExternalInput").ap()
    w1 = nc.dram_tensor("w1", [FC, P, DC, P], bf, kind="ExternalInput").ap()
    w2 = nc.dram_tensor("w2", [DC, P, FC, P], bf, kind="ExternalInput").ap()
    b1 = nc.dram_tensor("b1", [P, FC], f32, kind="ExternalInput").ap()
    b2 = nc.dram_tensor("b2", [P, DC], f32, kind="ExternalInput").ap()
    # bf16 output: halves the writeback DMA (host combines in fp32;
    # the extra ~0.2% quantization is far inside the 2e-2 gate)
    out = nc.dram_tensor("out", [D, CAP], bf, kind="ExternalOutput").ap()

    with tile.TileContext(nc) as tc, ExitStack() as ctx:
        cpool = ctx.enter_context(tc.tile_pool(name="consts", bufs=1))
        wpool = ctx.enter_context(tc.tile_pool(name="weights", bufs=1))
        xpool = ctx.enter_context(tc.tile_pool(name="xin", bufs=2))
        hpool = ctx.enter_context(tc.tile_pool(name="hmid", bufs=1))
        ypool = ctx.enter_context(tc.tile_pool(name="yout", bufs=4))
        # ph + py are distinct tags; 4 bufs each = all 8 PSUM banks
        ppool = ctx.enter_context(tc.tile_pool(name="psum", bufs=4, space="PSUM"))

        # PE warm-up: junk matmuls bridging the ~6us engine preamble plus
        # the first-DMA latency so the HAM clock-gate reaches 2.4 GHz and
        # the real stream starts warm with no idle gap (>~3.4us idle
        # would drop PE back to 1.2 GHz).
        warm = cpool.tile([P, NT], bf)
        nc.vector.memset(warm, 0.0)
        pwarm = ppool.tile([P, NT], f32, tag="ph")
        for _ in range(12):
            nc.tensor.matmul(pwarm, lhsT=warm[:, :P], rhs=warm, start=True,
                             stop=True)

        def load_x(t):
            xs = xpool.tile([P, DC, NT], bf, name=f"x_s{t}", tag="xs")
            nc.sync.dma_start(out=xs, in_=xT[t])
            return xs

        # Startup-critical DMAs split across the two HWDGE queues so
        # they land in parallel (each queue caps at ~150-195 GB/s while
        # ramping; together ~340).  The first matmul group needs all of
        # x(0) plus w1 block 0, so x(0) is split across the queues
        # roughly by their measured rates, and w1 block 0 leads the
        # Scalar queue (the tiny strided bias loads delayed that
        # queue's first packet by ~2us when they went first).  Block k
        # of w1 is consumed ~1.73us after block k-1 but transfers in
        # ~1.3us on the Sync queue alone, so the stream never waits
        # after the first group.
        x_cur = xpool.tile([P, DC, NT], bf, name="x_s0", tag="xs")
        XS = 6  # x(0) d-chunks on the Sync queue (rest on Scalar)
        w1_s = wpool.tile([P, FC, DC, P], bf)
        nc.sync.dma_start(out=x_cur[:, :XS, :], in_=xT[0, :, :XS, :])
        nc.scalar.dma_start(out=w1_s[:, 0], in_=w1[0])
        nc.scalar.dma_start(out=x_cur[:, XS:, :], in_=xT[0, :, XS:, :])
        nc.sync.dma_start(out=w1_s[:, 1], in_=w1[1])
        nc.scalar.dma_start(out=w1_s[:, 2], in_=w1[2])
        # the biases are 128 tiny descriptors each (~1.5us of queue
        # descriptor processing) - slot them here on Scalar: late enough
        # not to delay x(0)/fb0-2 (tried leading the queue: +2us stall),
        # early enough for the first gelu (needs b1 before psum
        # backpressure at ~stream_start+7us; tried trailing: re-stall)
        b1_s = cpool.tile([P, FC], f32)
        nc.scalar.dma_start(out=b1_s, in_=b1)
        b2_s = cpool.tile([P, DC], f32)
        nc.scalar.dma_start(out=b2_s, in_=b2)
        nc.sync.dma_start(out=w1_s[:, 3], in_=w1[3])
        nc.scalar.dma_start(out=w1_s[:, 4], in_=w1[4])
        nc.sync.dma_start(out=w1_s[:, 5], in_=w1[5])
        nc.scalar.dma_start(out=w1_s[:, 6], in_=w1[6])
        for fb in range(7, FC):
            nc.sync.dma_start(out=w1_s[:, fb], in_=w1[fb])
        w2_s = wpool.tile([P, DC, FC, P], bf)
        for dc in range(DC):
            nc.sync.dma_start(out=w2_s[:, dc], in_=w2[dc])

        for t in range(TT):
            x_s = x_cur
            if t + 1 < TT:
                x_cur = load_x(t + 1)
            # h.T tile [f, tok] for this token tile
            h_s = hpool.tile([P, FC, NT], bf)
            for fc in range(FC):
                ph = ppool.tile([P, NT], f32, tag="ph")
                for c in range(DC):
                    nc.tensor.matmul(
                        ph,
                        lhsT=w1_s[:, fc, c, :],
                        rhs=x_s[:, c, :],
                        start=(c == 0),
                        stop=(c == DC - 1),
                    )
                nc.scalar.activation(
                    h_s[:, fc, :], ph, AF.Gelu, bias=b1_s[:, fc:fc + 1]
                )
            for dc in range(DC):
                # split the kernel's final group so less serial ACT+DMA
                # trails the last matmul (the trailing chain scales with
                # the last part's width)
                last = t == TT - 1 and dc == DC - 1
                widths = [192, 192, 64, 64] if last else [NT]
                off = 0
                for s, w in enumerate(widths):
                    py = ppool.tile([P, NT], f32, name="py", tag="py")
                    for fc in range(FC):
                        nc.tensor.matmul(
                            py[:, :w],
                            lhsT=w2_s[:, dc, fc, :],
                            rhs=h_s[:, fc, off:off + w],
                            start=(fc == 0),
                            stop=(fc == FC - 1),
                        )
                    y_s = ypool.tile([P, NT], bf, name="y_s", tag="ys")
                    nc.scalar.activation(
                        y_s[:, :w], py[:, :w], AF.Identity,
                        bias=b2_s[:, dc:dc + 1]
                    )
                    # final group: alternate queues so its writeback
                    # drains in parallel; last part issues on the same
                    # engine as its ACT (no cross-engine sem hop)
                    eng = nc.scalar if (last and s % 2 == 1) else nc.sync
                    eng.dma_start(
                        out=out[dc * P:(dc + 1) * P,
                                t * NT + off:t * NT + off + w],
                        in_=y_s[:, :w],
                    )
                    off += w
    nc.compile()
    return nc


def _route(x, w_router, b_router):
    """Replicates reference routing (softmax -> top-2 -> capacity) in f64.

    Returns per-expert (token_ids, slot_positions, gate_values)."""
    xf = x.reshape(N, D).astype(np.float64)
    logits = xf @ w_router.astype(np.float64) + b_router.astype(np.float64)
    logits -= logits.max(axis=-1, keepdims=True)
    p = np.exp(logits)
    gates = p / p.sum(axis=-1, keepdims=True)
    # top-2, ties to the lower index (matches lax.top_k)
    order = np.argsort(-gates, axis=1, kind="stable")[:, :TOP_K]
    topv = np.take_along_axis(gates, order, axis=1)
    e_flat = order.reshape(-1)
    g_flat = topv.reshape(-1).astype(np.float32)
    tok = np.repeat(np.arange(N), TOP_K)
    pos = np.empty(N * TOP_K, np.int64)
    for e in range(E):
        m_e = e_flat == e
        pos[m_e] = np.arange(int(m_e.sum()))
    keep = pos < CAP
    per_expert = []
    for e in range(E):
        sel = (e_flat == e) & keep
        per_expert.append((tok[sel], pos[sel], g_flat[sel]))
    return per_expert


def _run_device(in_maps, trace=False):
    from concourse.bass_utils import run_bass_kernel_spmd

    if "nc" not in _NC_CACHE:
        _NC_CACHE["nc"] = _build_nc()
    return run_bass_kernel_spmd(
        _NC_CACHE["nc"], in_maps, core_ids=list(range(NCORES)), trace=trace
    )


def _kernel_impl(inputs, trace=False):
    x = np.asarray(inputs["x"], dtype=np.float32)
    w_router = np.asarray(inputs["w_router"], dtype=np.float32)
    b_router = np.asarray(inputs["b_router"], dtype=np.float32)
    w1 = np.asarray(inputs["w1"], dtype=np.float32)
    b1 = np.ascontiguousarray(np.asarray(inputs["b1"], dtype=np.float32))
    w2 = np.asarray(inputs["w2"], dtype=np.float32)
    b2 = np.ascontiguousarray(np.asarray(inputs["b2"], dtype=np.float32))

    per_expert = _route(x, w_router, b_router)
    xf = x.reshape(N, D)

    in_maps = []
    for e in range(E):
        tk, ps, _ = per_expert[e]
        buf = np.zeros((CAP, D), np.float32)
        buf[ps] = xf[tk]
        bufT = buf.T                                # [D, CAP]
        # stream-block layouts (see _build_nc)
        xp = np.ascontiguousarray(
            bufT.reshape(DC, P, TT, NT).transpose(2, 1, 0, 3)).astype(BF16)
        w1p = np.ascontiguousarray(
            w1[e].reshape(DC, P, FC, P).transpose(2, 1, 0, 3)).astype(BF16)
        w2p = np.ascontiguousarray(
            w2[e].reshape(FC, P, DC, P).transpose(2, 1, 0, 3)).astype(BF16)
        b1p = np.ascontiguousarray(b1[e].reshape(FC, P).T)
        b2p = np.ascontiguousarray(b2[e].reshape(DC, P).T)
        in_maps.append({
            "xT": xp, "w1": w1p, "w2": w2p, "b1": b1p, "b2": b2p,
        })

    res = _run_device(in_maps, trace=trace)

    y = np.zeros((N, D), np.float32)
    ws = np.zeros((N,), np.float32)
    for e in range(E):
        tk, ps, gv = per_expert[e]
        outT = np.asarray(res.results[e]["out"]).astype(np.float32)  # [D, CAP]
        vals = (outT[:, ps] * gv[None, :]).T  # [n_e, D]
        y[tk] += vals                         # tk unique within one expert
        ws[tk] += gv
    y = np.where((ws > 0.0)[:, None], y / np.maximum(ws, 1e-6)[:, None], y)
    return y.reshape(B, T, D).astype(np.float32), res


def kernel(**inputs):
    y, _ = _kernel_impl(inputs, trace=False)
    return y


# revision 11
# speedup vs baseline: 1.0094x; 1.0010x over previous
"""MoE MLP (top-2 routing, capacity 1.25) on 8 Trainium2 NeuronCores.

Strategy (expert-parallel, per the sharding hint):
  - Router + top-k + capacity assignment run on host in float64 (cheap:
    0.27 GFLOP vs 344 GFLOP for the expert FFNs, and data-dependent
    control flow is a poor fit for the static Bass dataflow graph).
  - Every expert overflows capacity for this problem size (mean load
    4096 assignments vs cap 2560), so each of the 8 cores computes a
    dense [cap,D] @ [D,F] -> gelu -> [cap,F] @ [F,D] FFN for one expert.
  - Dispatch/combine (gather/scatter by routing indices) run on host.

Device kernel: bf16 matmuls with both weight stacks resident in SBUF
(16 MB, fits), activations feature-major ([D,cap]/[F,cap]) so weight
tiles are always the stationary operand and no transposes are needed.
The 2560-MM stream runs at the warm PE issue rate (~216 ns/MM at
N=512), so everything else is startup/tail engineering:
  - all streamed tensors are host-prepacked into layouts where every
    DMA is linear with >=2 KB per-partition runs (small-packet DMA ran
    at ~175 GB/s vs ~350 GB/s for big packets),
  - startup-critical loads are split across both HWDGE queues (Sync +
    Activation) so x(0) and the first w1 column-blocks land in
    parallel; issue order keeps every later block ahead of the PE,
  - biases go over HWDGE as host-pretransposed [P, n] linear rows (the
    SWDGE path delivered them ~18 us in, stalling the first gelu),
  - junk warmup matmuls bridge the preamble+first-DMA window so HAM
    reaches 2.4 GHz with no re-throttle,
  - the final psum group is split in quarters so only ~1 us of serial
    ACT+DMA trails the last matmul.
bf16 keeps HBM traffic at ~27 MB/core (vs 190 MB for the f32r
variant), which also stays under the board power limit - the f32r run
spent 161 us GPIO-throttled to 1.95 GHz.  ~571.5 us, rel err ~3.8e-3
(gate 2e-2); breakdown: ~7 us engine preamble, ~5 us DMA ramp +
critical x/w1 transfer (overlapped by warmup), ~553 us matmul stream
at the 216 ns/MM warm issue rate (96.7% of bf16 peak), ~5 us tail
(final ACT+DMA latency + fixed teardown).
"""

import numpy as np
import ml_dtypes

B, T, D, F_FF, E, TOP_K = 8, 2048, 1024, 4096, 8, 2
N = B * T
CAP = 2560          # int(1.25 * N / E)
NCORES = 8
P = 128
DC = D // P         # 8 chunks of the model dim
FC = F_FF // P      # 32 chunks of the ff dim
NT = 512            # token tile (one PSUM bank of fp32)
TT = CAP // NT      # 5 token tiles

BF16 = ml_dtypes.bfloat16

MODE = "bf16"       # kept for test.py compatibility; single path now

_NC_CACHE = {}


def _build_nc():
    """Per-core Bass graph: dense bf16 FFN for one expert (SPMD x8)."""
    from contextlib import ExitStack

    import concourse.mybir as mybir
    import concourse.tile as tile
    from concourse import bacc

    bf = mybir.dt.bfloat16
    f32 = mybir.dt.float32
    AF = mybir.ActivationFunctionType

    nc = bacc.Bacc(trn_type="TRN2")
    # host-prepacked layouts (all loads below are linear copies):
    #   xT[t,p,c,n]  = x.T chunk: token tile t, d-row c*128+p, token n
    #   w1[fb,p,c,j] = w1[c*128+p, fb*128+j]   (128-col blocks of [D,F])
    #   w2[dc,p,f,j] = w2[f*128+p, dc*128+j]   (128-col blocks of [F,D])
    #   b1[p,fc] = b1[fc*128+p]; b2[p,dc] = b2[dc*128+p]
    xT = nc.dram_tensor("xT", [TT, P, DC, NT], bf, kind="ExternalInput").ap()
    w1 = nc.dram_tensor("w1", [FC, P, DC, P], bf, kind="ExternalInput").ap()
    w2 = nc.dram_tensor("w2", [DC, P, FC, P], bf, kind="ExternalInput").ap()
    b1 = nc.dram_tensor("b1", [P, FC], f32, kind="ExternalInput").ap()
    b2 = nc.dram_tensor("b2", [P, DC], f32, kind="ExternalInput").ap()
    # bf16 output: halves the writeback DMA (host combines in fp32;
    # the extra ~0.2% quantization is far inside the 2e-2 gate)
    out = nc.dram_tensor("out", [D, CAP], bf, kind="ExternalOutput").ap()

    with tile.TileContext(nc) as tc, ExitStack() as ctx:
        cpool = ctx.enter_context(tc.tile_pool(name="consts", bufs=1))
        wpool = ctx.enter_context(tc.tile_pool(name="weights", bufs=1))
        xpool = ctx.enter_context(tc.tile_pool(name="xin", bufs=2))
        hpool = ctx.enter_context(tc.tile_pool(name="hmid", bufs=1))
        ypool = ctx.enter_context(tc.tile_pool(name="yout", bufs=4))
        # ph + py are distinct tags; 4 bufs each = all 8 PSUM banks
        ppool = ctx.enter_context(tc.tile_pool(name="psum", bufs=4, space="PSUM"))

        # PE warm-up: junk matmuls bridging the ~6us engine preamble plus
        # the first-DMA latency so the HAM clock-gate reaches 2.4 GHz and
        # the real stream starts warm with no idle gap (>~3.4us idle
        # would drop PE back to 1.2 GHz).
        warm = cpool.tile([P, NT], bf)
        nc.vector.memset(warm, 0.0)
        pwarm = ppool.tile([P, NT], f32, tag="ph")
        for _ in range(12):
            nc.tensor.matmul(pwarm, lhsT=warm[:, :P], rhs=warm, start=True,
                             stop=True)

        def load_x(t):
            xs = xpool.tile([P, DC, NT], bf, name=f"x_s{t}", tag="xs")
            nc.sync.dma_start(out=xs, in_=xT[t])
            return xs

        # Startup-critical DMAs split across the two HWDGE queues so
        # they land in parallel (each queue caps at ~150-195 GB/s while
        # ramping; together ~340).  The first matmul group needs all of
        # x(0) plus w1 block 0, so x(0) is split across the queues
        # roughly by their measured rates, and w1 block 0 leads the
        # Scalar queue (the tiny strided bias loads delayed that
        # queue's first packet by ~2us when they went first).  Block k
        # of w1 is consumed ~1.73us after block k-1 but transfers in
        # ~1.3us on the Sync queue alone, so the stream never waits
        # after the first group.
        x_cur = xpool.tile([P, DC, NT], bf, name="x_s0", tag="xs")
        XS = 6  # x(0) d-chunks on the Sync queue (rest on Scalar)
        w1_s = wpool.tile([P, FC, DC, P], bf)
        nc.sync.dma_start(out=x_cur[:, :XS, :], in_=xT[0, :, :XS, :])
        nc.scalar.dma_start(out=w1_s[:, 0], in_=w1[0])
        nc.scalar.dma_start(out=x_cur[:, XS:, :], in_=xT[0, :, XS:, :])
        nc.sync.dma_start(out=w1_s[:, 1], in_=w1[1])
        nc.scalar.dma_start(out=w1_s[:, 2], in_=w1[2])
        # the biases are 128 tiny descriptors each (~1.5us of queue
        # descriptor processing) - slot them here on Scalar: late enough
        # not to delay x(0)/fb0-2 (tried leading the queue: +2us stall),
        # early enough for the first gelu (needs b1 before psum
        # backpressure at ~stream_start+7us; tried trailing: re-stall)
        b1_s = cpool.tile([P, FC], f32)
        nc.scalar.dma_start(out=b1_s, in_=b1)
        b2_s = cpool.tile([P, DC], f32)
        nc.scalar.dma_start(out=b2_s, in_=b2)
        nc.sync.dma_start(out=w1_s[:, 3], in_=w1[3])
        nc.scalar.dma_start(out=w1_s[:, 4], in_=w1[4])
        nc.sync.dma_start(out=w1_s[:, 5], in_=w1[5])
        nc.scalar.dma_start(out=w1_s[:, 6], in_=w1[6])
        for fb in range(7, FC):
            nc.sync.dma_start(out=w1_s[:, fb], in_=w1[fb])
        w2_s = wpool.tile([P, DC, FC, P], bf)
        for dc in range(DC):
            nc.sync.dma_start(out=w2_s[:, dc], in_=w2[dc])

        for t in range(TT):
            x_s = x_cur
            if t + 1 < TT:
                x_cur = load_x(t + 1)
            # h.T tile [f, tok] for this token tile
            h_s = hpool.tile([P, FC, NT], bf)
            for fc in range(FC):
                ph = ppool.tile([P, NT], f32, tag="ph")
                for c in range(DC):
                    nc.tensor.matmul(
                        ph,
                        lhsT=w1_s[:, fc, c, :],
                        rhs=x_s[:, c, :],
                        start=(c == 0),
                        stop=(c == DC - 1),
                    )
                nc.scalar.activation(
                    h_s[:, fc, :], ph, AF.Gelu, bias=b1_s[:, fc:fc + 1]
                )
            for dc in range(DC):
                # split the kernel's final group so less serial ACT+DMA
                # trails the last matmul (the trailing chain scales with
                # the last part's width)
                last = t == TT - 1 and dc == DC - 1
                widths = [192, 192, 64, 64] if last else [NT]
                off = 0
                for s, w in enumerate(widths):
                    py = ppool.tile([P, NT], f32, name="py", tag="py")
                    for fc in range(FC):
                        nc.tensor.matmul(
                            py[:, :w],
                            lhsT=w2_s[:, dc, fc, :],
                            rhs=h_s[:, fc, off:off + w],
                            start=(fc == 0),
                            stop=(fc == FC - 1),
                        )
                    y_s = ypool.tile([P, NT], bf, name="y_s", tag="ys")
                    nc.scalar.activation(
                        y_s[:, :w], py[:, :w], AF.Identity,
                        bias=b2_s[:, dc:dc + 1]
                    )
                    # final group: alternate queues so its writeback
                    # drains in parallel; last part issues on the same
                    # engine as its ACT (no cross-engine sem hop)
                    eng = nc.scalar if (last and s % 2 == 1) else nc.sync
                    eng.dma_start(
                        out=out[dc * P:(dc + 1) * P,
                                t * NT + off:t * NT + off + w],
                        in_=y_s[:, :w],
                    )
                    off += w
    nc.compile()
    return nc


def _route(x, w_router, b_router):
    """Replicates reference routing (softmax -> top-2 -> capacity) in f64.

    Returns per-expert (token_ids, slot_positions, gate_values)."""
    xf = x.reshape(N, D).astype(np.float64)
    logits = xf @ w_router.astype(np.float64) + b_router.astype(np.float64)
    logits -= logits.max(axis=-1, keepdims=True)
    p = np.exp(logits)
    gates = p / p.sum(axis=-1, keepdims=True)
    # top-2, ties to the lower index (matches lax.top_k)
    order = np.argsort(-gates, axis=1, kind="stable")[:, :TOP_K]
    topv = np.take_along_axis(gates, order, axis=1)
    e_flat = order.reshape(-1)
    g_flat = topv.reshape(-1).astype(np.float32)
    tok = np.repeat(np.arange(N), TOP_K)
    pos = np.empty(N * TOP_K, np.int64)
    for e in range(E):
        m_e = e_flat == e
        pos[m_e] = np.arange(int(m_e.sum()))
    keep = pos < CAP
    per_expert = []
    for e in range(E):
        sel = (e_flat == e) & keep
        per_expert.append((tok[sel], pos[sel], g_flat[sel]))
    return per_expert


def _run_device(in_maps, trace=False):
    from concourse.bass_utils import run_bass_kernel_spmd

    if "nc" not in _NC_CACHE:
        _NC_CACHE["nc"] = _build_nc()
    return run_bass_kernel_spmd(
        _NC_CACHE["nc"], in_maps, core_ids=list(range(NCORES)), trace=trace
    )


def _kernel_impl(inputs, trace=False):
    x = np.asarray(inputs["x"], dtype=np.float32)
    w_router = np.asarray(inputs["w_router"], dtype=np.float32)
    b_router = np.asarray(inputs["b_router"], dtype=np.float32)
    w1 = np.asarray(inputs["w1"], dtype=np.float32)
    b1 = np.ascontiguousarray(np.asarray(inputs["b1"], dtype=np.float32))
    w2 = np.asarray(inputs["w2"], dtype=np.float32)
    b2 = np.ascontiguousarray(np.asarray(inputs["b2"], dtype=np.float32))

    per_expert = _route(x, w_router, b_router)
    xf = x.reshape(N, D)

    in_maps = []
    for e in range(E):
        tk, ps, _ = per_expert[e]
        buf = np.zeros((CAP, D), np.float32)
        buf[ps] = xf[tk]
        bufT = buf.T                                # [D, CAP]
        # stream-block layouts (see _build_nc)
        xp = np.ascontiguousarray(
            bufT.reshape(DC, P, TT, NT).transpose(2, 1, 0, 3)).astype(BF16)
        w1p = np.ascontiguousarray(
            w1[e].reshape(DC, P, FC, P).transpose(2, 1, 0, 3)).astype(BF16)
        w2p = np.ascontiguousarray(
            w2[e].reshape(FC, P, DC, P).transpose(2, 1, 0, 3)).astype(BF16)
        b1p = np.ascontiguousarray(b1[e].reshape(FC, P).T)
        b2p = np.ascontiguousarray(b2[e].reshape(DC, P).T)
        in_maps.append({
            "xT": xp, "w1": w1p, "w2": w2p, "b1": b1p, "b2": b2p,
        })

    res = _run_device(in_maps, trace=trace)

    y = np.zeros((N, D), np.float32)
    ws = np.zeros((N,), np.float32)
    for e in range(E):
        tk, ps, gv = per_expert[e]
        outT = np.asarray(res.results[e]["out"]).astype(np.float32)  # [D, CAP]
        vals = (outT[:, ps] * gv[None, :]).T  # [n_e, D]
        y[tk] += vals                         # tk unique within one expert
        ws[tk] += gv
    y = np.where((ws > 0.0)[:, None], y / np.maximum(ws, 1e-6)[:, None], y)
    return y.reshape(B, T, D).astype(np.float32), res


def kernel(**inputs):
    y, _ = _kernel_impl(inputs, trace=False)
    return y
